# revision 26
# baseline (speedup 1.0000x reference)
"""Causal self-attention (B=2, T=4096, C=768, H=12, D=64) on 8 trn2 cores.

Sharding: batch*heads across cores. Core c handles batch c//4 and heads
3*(c%4) .. 3*(c%4)+2. Each core computes the QKV projection for its head
slice, full causal attention for those heads, and a partial output
projection (its heads' rows of w_out). The host sums the 4 partials per
batch and adds b_out.

All matmuls use a full K=128 contraction: the TRN2 PE runs K<=64
matmuls in a half-array "h64 row group" mode, and switching between
h64 and full mode flushes the PE pipeline (~650ns/switch, measured).
The baseline alternated K=64 scores with K=128 PV matmuls and paid two
switches per k-tile; here kT is stored zero-padded to 128 partitions
so scores contract over 128 rows (64 real + 64 zeros) and every matmul
runs in full mode at 1 cycle/row.

On-core layouts (matmul operands float32r - fp32 data consumed at full
PE rate with ~1e-4 rounding):
  xT      [C, T]   input, pre-transposed on host
  tA      [128,T]  [qT_h0 | qT_h1] (rows 0:64 | 64:128)
  tD      [128,T]  [outT_h2 | qT_h2]
  tC      [128,T]  [outT_h0 | outT_h1]
  kB[h]   [128,T]  kT_h zero-padded: h0 rows 0:64 + zeros; h1/h2 zeros
                   + rows 64:128 (aligned to where qT_h lives in its
                   moving-operand tile; the zero rows annihilate the
                   other head's q values that share that tile)
  v_aug   [T, 256] natural-layout v with a ones column per head at
                   col h*65+64 (so P@V also yields softmax denominators)
  scoresT [k, q]   psum; exp on ACT; causal mask via gpsimd affine_select
  outT    [65, q]  psum accumulation over k tiles; row 64 = sum(exp)

The output projection contracts tC (128 rows: h0|h1) and tD (rows 0:64
= h2 out, rows 64:128 = qT_h2 junk annihilated by zero rows 64:128 of
the padded w_out input, which the host supplies as [256, C]).

The projection is emitted chunk-by-chunk inside the attention q-block
loop, and the output projection for block qb is deferred until after
the attention of block qb+1 so its normalize/reciprocal dependency
chain (DVE reciprocal + DRAM-bounce partition broadcast) is off the PE
critical path.
"""

import numpy as np

import concourse.bass as bass
import concourse.mybir as mybir
import concourse.tile as tile
from concourse import bacc
from concourse.bass_utils import run_bass_kernel_spmd

B, T, C = 2, 4096, 768
NH, D = 12, 64
HPC = 3  # heads per core
NCORES = 8
P = 128
QB = 512           # q block == projection chunk
NQB = T // QB      # 8
NKT = T // P       # 32 k tiles
F32 = mybir.dt.float32
F32R = mybir.dt.float32r
WQK_COLS = 2 * HPC * D  # 3 projection chains of 128 output rows each

_CACHE = {}


def _build_nc():
    nc = bacc.Bacc(
        "TRN2",
        target_bir_lowering=False,
        debug=False,
        enable_asserts=False,
        num_devices=NCORES,
    )
    # wqk columns: [q_h0 q_h1 | k_h0 k_h1 | q_h2 | k_h2]
    xT = nc.dram_tensor("xT", [C, T], F32R, kind="ExternalInput")
    wqk = nc.dram_tensor("wqk", [C, WQK_COLS], F32R, kind="ExternalInput")
    wv = nc.dram_tensor("wv", [C, 256], F32R, kind="ExternalInput")
    wo = nc.dram_tensor("wo", [2 * P, C], F32R, kind="ExternalInput")
    out = nc.dram_tensor("out", [T, C], F32, kind="ExternalOutput")

    with tile.TileContext(nc) as tc:
        _emit(tc, nc, xT.ap(), wqk.ap(), wv.ap(), wo.ap(), out.ap())
    nc.compile()
    return nc


def _emit(tc, nc, xT, wqk, wv, wo, out):
    import contextlib

    ctx = contextlib.ExitStack()
    with ctx:
        # ---- persistent sbuf ----
        persist = ctx.enter_context(tc.tile_pool(name="persist", bufs=1))
        tA = persist.tile([P, T], F32R, tag="pkA", name="pkA")
        tC = persist.tile([P, T], F32R, tag="pkC", name="pkC")
        tD = persist.tile([P, T], F32R, tag="pkD", name="pkD")
        kB = [
            persist.tile([P, T], F32R, tag=f"kB{h}", name=f"kB{h}")
            for h in range(HPC)
        ]
        vaug = persist.tile([P, NKT, 256], F32R, tag="vaug")
        wqk_sb = persist.tile([P, 6, WQK_COLS], F32R, tag="wqk")
        wv_sb = persist.tile([P, 6, 256], F32R, tag="wv")
        wo01_sb = persist.tile([P, C], F32R, tag="wo01")
        wo2_sb = persist.tile([P, C], F32R, tag="wo2")
        ones_f32 = persist.tile([P, D], F32, tag="onesf32")
        zeros_f32 = persist.tile([P, QB], F32, tag="zerosf32")

        nc.sync.dma_start(out=wqk_sb[:], in_=wqk.rearrange("(co p) n -> p co n", p=P))
        nc.sync.dma_start(out=wv_sb[:], in_=wv.rearrange("(co p) n -> p co n", p=P))
        nc.sync.dma_start(out=wo01_sb[:], in_=wo[0:P, :])
        nc.sync.dma_start(out=wo2_sb[:], in_=wo[P : 2 * P, :])
        nc.gpsimd.memset(ones_f32[:], 1.0)
        nc.gpsimd.memset(zeros_f32[:], 0.0)
        # zero halves of the padded k tiles (never written elsewhere);
        # gpsimd memset cannot target f32r, so copy-cast from an f32
        # zero tile on DVE
        for cb in range(NQB):
            csl = slice(cb * QB, (cb + 1) * QB)
            nc.vector.tensor_copy(out=kB[0][D:P, csl], in_=zeros_f32[D:P, :])
            nc.vector.tensor_copy(out=kB[1][0:D, csl], in_=zeros_f32[0:D, :])
            nc.vector.tensor_copy(out=kB[2][0:D, csl], in_=zeros_f32[0:D, :])

        def qmv(h):
            # moving operand tile for head h's scores (full 128 rows)
            return (tA, tA, tD)[h]

        # ---- fused projection + attention loop ----
        # psum budget (8 banks): p1 2 + scores 3 + p3 1 + outT 2
        with (
            tc.tile_pool(name="xchunks", bufs=2) as xpool,
            tc.tile_pool(name="p1psum", bufs=2, space="PSUM") as p1psum,
            tc.tile_pool(name="spsum", bufs=3, space="PSUM") as spool,
            tc.tile_pool(name="p3psum", bufs=1, space="PSUM") as p3psum,
            tc.tile_pool(name="opsum", bufs=2, space="PSUM") as opool,
            tc.tile_pool(name="exps", bufs=3) as epool,
            tc.tile_pool(name="smalls", bufs=4) as rpool,
            tc.tile_pool(name="dscratch", bufs=4, space="DRAM") as dpool,
        ):
            def emit_outproj(qb):
                # output projection for q block qb (tail of the loop;
                # psum comes from its own 1-bank pool)
                for tt in range(qb * (QB // P), (qb + 1) * (QB // P)):
                    tsl = slice(tt * P, (tt + 1) * P)
                    so = rpool.tile([P, C], F32, tag="p3out", bufs=2)
                    for noff, nsz in ((0, 512), (512, 256)):
                        po = p3psum.tile(
                            [P, QB], F32, tag="p3", name=f"po_{tt}_{noff}"
                        )
                        nc.tensor.matmul(
                            po[:, :nsz],
                            tC[:, tsl],
                            wo01_sb[:, noff : noff + nsz],
                            start=True,
                            stop=False,
                        )
                        nc.tensor.matmul(
                            po[:, :nsz],
                            tD[:, tsl],
                            wo2_sb[:, noff : noff + nsz],
                            start=False,
                            stop=True,
                        )
                        nc.vector.tensor_copy(
                            out=so[:, noff : noff + nsz], in_=po[:, :nsz]
                        )
                    nc.sync.dma_start(out=out[tsl, :], in_=so[:])

            for qb in range(NQB):
                qsl = slice(qb * QB, (qb + 1) * QB)

                # -- projection chunk qb: columns [qb*512, qb*512+512) --
                xt = xpool.tile([P, 6, QB], F32R, tag="xt", name=f"xt{qb}")
                nc.sync.dma_start(
                    out=xt[:], in_=xT[:, qsl].rearrange("(co p) t -> p co t", p=P)
                )
                for ci in range(3):
                    ps = p1psum.tile([P, QB], F32, tag="p1", name=f"p1_{qb}_{ci}")
                    for c6 in range(6):
                        nc.tensor.matmul(
                            ps[:],
                            wqk_sb[:, c6, ci * P : (ci + 1) * P],
                            xt[:, c6, :],
                            start=(c6 == 0),
                            stop=(c6 == 5),
                        )
                    if ci == 0:
                        # [q_h0 | q_h1]
                        nc.vector.tensor_copy(out=tA[:, qsl], in_=ps[:])
                    elif ci == 1:
                        # [k_h0 | k_h1] -> split into padded kB tiles
                        # (aligned partition ranges; no base crossing)
                        nc.vector.tensor_copy(out=kB[0][0:D, qsl], in_=ps[0:D, :])
                        nc.vector.tensor_copy(out=kB[1][D:P, qsl], in_=ps[D:P, :])
                    else:
                        # [q_h2 | k_h2]: k_h2 lands aligned; q_h2 must
                        # cross to partition base 64 of tD, which only a
                        # DMA can do (engines cannot cross partitions)
                        nc.vector.tensor_copy(out=kB[2][D:P, qsl], in_=ps[D:P, :])
                        stg = xpool.tile([D, QB], F32R, tag="stg")
                        nc.vector.tensor_copy(out=stg[:], in_=ps[0:D, :])
                        nc.sync.dma_start(out=tD[D:P, qsl], in_=stg[:])
                for half in range(QB // P):
                    ktv = qb * (QB // P) + half
                    ps2 = p1psum.tile([P, QB], F32, tag="p1", name=f"p1v_{qb}_{half}")
                    for c6 in range(6):
                        nc.tensor.matmul(
                            ps2[:, 0:256],
                            xt[:, c6, half * P : (half + 1) * P],
                            wv_sb[:, c6, :],
                            start=(c6 == 0),
                            stop=(c6 == 5),
                        )
                    nc.vector.tensor_copy(out=vaug[:, ktv, :], in_=ps2[:, 0:256])
                # restore the ones columns the v copies just overwrote
                for h in range(HPC):
                    nc.vector.tensor_copy(
                        out=vaug[:, qb * (QB // P) : (qb + 1) * (QB // P),
                                 h * (D + 1) + D],
                        in_=ones_f32[:, 0 : QB // P],
                    )

                # -- attention for q block qb --
                # rows 0:64 of tD (outT_h2) are read by h2's padded-K
                # scores before they are first written; zero them so
                # stray NaN/Inf bit patterns cannot poison 0*x products
                nc.vector.tensor_copy(out=tD[0:D, qsl], in_=zeros_f32[0:D, :])
                for h in range(HPC):
                    nkt = 4 * qb + 4
                    outp = opool.tile([D + 1, QB], F32, tag="outT")
                    for kt in range(nkt):
                        ktr = kt - 4 * qb
                        # clamp co to 256: f32r matmuls with moving dim
                        # <256 drop to 1/4 rate, so N=128 costs more
                        # than N=256 (extra columns are masked anyway)
                        co = min(max(0, P * ktr), 256)
                        sp = spool.tile([P, QB], F32, tag="scores")
                        nc.tensor.matmul(
                            sp[:, co:],
                            kB[h][:, kt * P : (kt + 1) * P],
                            qmv(h)[:, qb * QB + co : (qb + 1) * QB],
                            start=True,
                            stop=True,
                        )
                        ex = epool.tile([P, QB], F32R, tag="ex")
                        nc.scalar.activation(
                            out=ex[:, co:],
                            in_=sp[:, co:],
                            func=mybir.ActivationFunctionType.Exp,
                            scale=float(D) ** -0.5,
                        )
                        if ktr >= 0:  # diagonal band: causal mask
                            nc.gpsimd.affine_select(
                                out=ex[:, co:],
                                in_=ex[:, co:],
                                compare_op=mybir.AluOpType.is_ge,
                                fill=0.0,
                                base=co - ktr * P,
                                pattern=[[1, QB - co]],
                                channel_multiplier=-1,
                            )
                        nc.tensor.matmul(
                            outp[:, co:],
                            vaug[:, kt, h * (D + 1) : (h + 1) * (D + 1)],
                            ex[:, co:],
                            start=(kt == 0),
                            stop=(kt == nkt - 1),
                        )
                    # softmax denominators: reciprocal of outp row 64 stays at
                    # partition base 64 (engines cannot cross partitions); a
                    # partition-broadcast DMA then fans it out across 0:64
                    recip = rpool.tile([D + 1, QB], F32, tag="recip", bufs=3)
                    nc.vector.reciprocal(
                        out=recip[D : D + 1, :], in_=outp[D : D + 1, :]
                    )
                    # partition-broadcast via DRAM bounce (SBUF sources must
                    # have nonzero partition step; DRAM reads may broadcast)
                    dsc = dpool.tile([1, QB], F32, tag="dsc")
                    nc.sync.dma_start(out=dsc[:], in_=recip[D : D + 1, :])
                    bcs = rpool.tile([D, QB], F32, tag="bcs", bufs=3)
                    nc.gpsimd.dma_start(
                        out=bcs[:],
                        in_=bass.AP(
                            tensor=dsc.tensor,
                            offset=dsc.offset,
                            ap=[[0, D]] + list(dsc.ap[-1:]),
                        ),
                    )
                    if h == 0:
                        nc.vector.tensor_mul(
                            out=tC[0:D, qsl], in0=outp[0:D, :], in1=bcs[:]
                        )
                    elif h == 2:
                        nc.vector.tensor_mul(
                            out=tD[0:D, qsl], in0=outp[0:D, :], in1=bcs[:]
                        )
                    else:
                        # h1 lives at partition base 64 of tC; engines cannot
                        # cross partitions, so normalize into a staging tile
                        # and DMA-bounce it up
                        ot = rpool.tile([D, QB], F32R, tag="otmp", bufs=2)
                        nc.vector.tensor_mul(
                            out=ot[:], in0=outp[0:D, :], in1=bcs[:]
                        )
                        nc.sync.dma_start(out=tC[D:P, qsl], in_=ot[:])

                # deferred output projection: emit block qb-1 now so its
                # normalize chain resolved during this block's attention
                if qb > 0:
                    emit_outproj(qb - 1)
            emit_outproj(NQB - 1)


def _get_nc():
    if "nc" not in _CACHE:
        _CACHE["nc"] = _build_nc()
    return _CACHE["nc"]


def _shard_inputs(x, w_qkv, w_out):
    """Build per-core input maps."""
    x = np.asarray(x, dtype=np.float32)
    w_qkv = np.asarray(w_qkv, dtype=np.float32)
    w_out = np.asarray(w_out, dtype=np.float32)
    xTs = [np.ascontiguousarray(x[b].T) for b in range(B)]
    in_maps = []
    for c in range(NCORES):
        b = c // 4
        heads = [HPC * (c % 4) + i for i in range(HPC)]
        q = [w_qkv[:, h * D : (h + 1) * D] for h in heads]
        k = [w_qkv[:, C + h * D : C + (h + 1) * D] for h in heads]
        wqk = np.concatenate([q[0], q[1], k[0], k[1], q[2], k[2]], axis=1)
        wv = np.zeros((C, 256), dtype=np.float32)
        for i, h in enumerate(heads):
            wv[:, i * (D + 1) : i * (D + 1) + D] = w_qkv[
                :, 2 * C + h * D : 2 * C + (h + 1) * D
            ]
        # [h0 rows | h1 rows | h2 rows | 64 zero rows]: rows 192:256
        # annihilate the qT_h2 junk in rows 64:128 of tD during the
        # output projection's second (tD) contraction
        wo = np.concatenate(
            [w_out[h * D : (h + 1) * D, :] for h in heads]
            + [np.zeros((D, C), dtype=np.float32)],
            axis=0,
        )
        in_maps.append(
            {
                "xT": xTs[b],
                "wqk": np.ascontiguousarray(wqk),
                "wv": wv,
                "wo": np.ascontiguousarray(wo),
            }
        )
    return in_maps


def kernel(x, w_qkv, w_out, b_out):
    nc = _get_nc()
    in_maps = _shard_inputs(x, w_qkv, w_out)
    res = run_bass_kernel_spmd(nc, in_maps, core_ids=list(range(NCORES)))
    b_out = np.asarray(b_out, dtype=np.float32)
    outs = []
    for b in range(B):
        acc = res.results[4 * b]["out"].astype(np.float32).copy()
        for c in range(4 * b + 1, 4 * b + 4):
            acc += res.results[c]["out"]
        outs.append(acc + b_out[None, :])
    return np.stack(outs, axis=0)


# revision 40
# speedup vs baseline: 1.0120x; 1.0120x over previous
"""Causal self-attention (B=2, T=4096, C=768, H=12, D=64) on 8 trn2 cores.

Sharding: batch*heads across cores. Core c handles batch c//4 and heads
3*(c%4) .. 3*(c%4)+2. Each core computes the QKV projection for its head
slice, full causal attention for those heads, and a partial output
projection (its heads' rows of w_out). The host sums the 4 partials per
batch and adds b_out.

All matmuls use a full K=128 contraction: the TRN2 PE runs K<=64
matmuls in a half-array "h64 row group" mode, and switching between
h64 and full mode flushes the PE pipeline (~650ns/switch, measured).
The baseline alternated K=64 scores with K=128 PV matmuls and paid two
switches per k-tile; here kT is stored zero-padded to 128 partitions
so scores contract over 128 rows (64 real + 64 zeros) and every matmul
runs in full mode at 1 cycle/row.

On-core layouts (matmul operands float32r - fp32 data consumed at full
PE rate with ~1e-4 rounding):
  xT      [C, T]   input, pre-transposed on host
  tA      [128,T]  [qT_h0 | qT_h1] (rows 0:64 | 64:128)
  tD      [128,T]  [outT_h2 | qT_h2]
  tC      [128,T]  [outT_h0 | outT_h1]
  kB[h]   [128,T]  kT_h zero-padded: h0 rows 0:64 + zeros; h1/h2 zeros
                   + rows 64:128 (aligned to where qT_h lives in its
                   moving-operand tile; the zero rows annihilate the
                   other head's q values that share that tile)
  v_aug   [T, 256] natural-layout v with a ones column per head at
                   col h*65+64 (so P@V also yields softmax denominators)
  scoresT [k, q]   psum; exp on ACT; causal mask via gpsimd affine_select
  outT    [65, q]  psum accumulation over k tiles; row 64 = sum(exp)

The output projection contracts tC (128 rows: h0|h1) and tD (rows 0:64
= h2 out, rows 64:128 = qT_h2 junk annihilated by zero rows 64:128 of
the padded w_out input, which the host supplies as [256, C]).

The projection is emitted chunk-by-chunk inside the attention q-block
loop, and the output projection for block qb is deferred until after
the attention of block qb+1 so its normalize/reciprocal dependency
chain (DVE reciprocal + DRAM-bounce partition broadcast) is off the PE
critical path.
"""

import numpy as np

import concourse.bass as bass
import concourse.mybir as mybir
import concourse.tile as tile
from concourse import bacc
from concourse.bass_utils import run_bass_kernel_spmd

B, T, C = 2, 4096, 768
NH, D = 12, 64
HPC = 3  # heads per core
NCORES = 8
P = 128
QB = 512           # q block == projection chunk
NQB = T // QB      # 8
NKT = T // P       # 32 k tiles
F32 = mybir.dt.float32
F32R = mybir.dt.float32r
BF16 = mybir.dt.bfloat16
WQK_COLS = 2 * HPC * D  # 3 projection chains of 128 output rows each

_CACHE = {}


def _build_nc():
    nc = bacc.Bacc(
        "TRN2",
        target_bir_lowering=False,
        debug=False,
        enable_asserts=False,
        num_devices=NCORES,
    )
    # wqk columns: [q_h0 q_h1 | k_h0 k_h1 | q_h2 | k_h2]
    xT = nc.dram_tensor("xT", [C, T], F32R, kind="ExternalInput")
    wqk = nc.dram_tensor("wqk", [C, WQK_COLS], F32R, kind="ExternalInput")
    wv = nc.dram_tensor("wv", [C, 256], F32R, kind="ExternalInput")
    wo = nc.dram_tensor("wo", [2 * P, C], F32R, kind="ExternalInput")
    out = nc.dram_tensor("out", [T, C], F32, kind="ExternalOutput")

    with tile.TileContext(nc) as tc:
        _emit(tc, nc, xT.ap(), wqk.ap(), wv.ap(), wo.ap(), out.ap())
    nc.compile()
    return nc


def _emit(tc, nc, xT, wqk, wv, wo, out):
    import contextlib

    ctx = contextlib.ExitStack()
    with ctx:
        # ---- persistent sbuf ----
        persist = ctx.enter_context(tc.tile_pool(name="persist", bufs=1))
        tA = persist.tile([P, T], F32R, tag="pkA", name="pkA")
        tC = persist.tile([P, T], F32R, tag="pkC", name="pkC")
        tD = persist.tile([P, T], F32R, tag="pkD", name="pkD")
        kB = [
            persist.tile([P, T], F32R, tag=f"kB{h}", name=f"kB{h}")
            for h in range(HPC)
        ]
        vaug = persist.tile([P, NKT, 256], F32R, tag="vaug")
        wqk_sb = persist.tile([P, 6, WQK_COLS], F32R, tag="wqk")
        wv_sb = persist.tile([P, 6, 256], F32R, tag="wv")
        wo01_sb = persist.tile([P, C], F32R, tag="wo01")
        wo2_sb = persist.tile([P, C], F32R, tag="wo2")
        ones_f32 = persist.tile([P, D], F32, tag="onesf32")
        zeros_f32 = persist.tile([P, QB], F32, tag="zerosf32")

        nc.sync.dma_start(out=wqk_sb[:], in_=wqk.rearrange("(co p) n -> p co n", p=P))
        nc.sync.dma_start(out=wv_sb[:], in_=wv.rearrange("(co p) n -> p co n", p=P))
        nc.sync.dma_start(out=wo01_sb[:], in_=wo[0:P, :])
        nc.sync.dma_start(out=wo2_sb[:], in_=wo[P : 2 * P, :])
        nc.gpsimd.memset(ones_f32[:], 1.0)
        nc.gpsimd.memset(zeros_f32[:], 0.0)
        # zero halves of the padded k tiles (never written elsewhere);
        # gpsimd memset cannot target f32r, so copy-cast from an f32
        # zero tile on DVE
        for cb in range(NQB):
            csl = slice(cb * QB, (cb + 1) * QB)
            nc.vector.tensor_copy(out=kB[0][D:P, csl], in_=zeros_f32[D:P, :])
            nc.vector.tensor_copy(out=kB[1][0:D, csl], in_=zeros_f32[0:D, :])
            nc.vector.tensor_copy(out=kB[2][0:D, csl], in_=zeros_f32[0:D, :])

        def qmv(h):
            # moving operand tile for head h's scores (full 128 rows)
            return (tA, tA, tD)[h]

        # ---- fused projection + attention loop ----
        # psum budget (8 banks): p1 2 + scores 3 + p3 1 + outT 2
        with (
            tc.tile_pool(name="xchunks", bufs=2) as xpool,
            tc.tile_pool(name="p1psum", bufs=2, space="PSUM") as p1psum,
            tc.tile_pool(name="spsum", bufs=3, space="PSUM") as spool,
            tc.tile_pool(name="p3psum", bufs=1, space="PSUM") as p3psum,
            tc.tile_pool(name="opsum", bufs=2, space="PSUM") as opool,
            tc.tile_pool(name="exps", bufs=3) as epool,
            tc.tile_pool(name="smalls", bufs=4) as rpool,
            tc.tile_pool(name="dscratch", bufs=4, space="DRAM") as dpool,
        ):
            def emit_outproj(qb):
                # output projection for q block qb (tail of the loop;
                # psum comes from its own 1-bank pool)
                for tt in range(qb * (QB // P), (qb + 1) * (QB // P)):
                    tsl = slice(tt * P, (tt + 1) * P)
                    so = rpool.tile([P, C], F32, tag="p3out", bufs=2)
                    for noff, nsz in ((0, 512), (512, 256)):
                        po = p3psum.tile(
                            [P, QB], F32, tag="p3", name=f"po_{tt}_{noff}"
                        )
                        nc.tensor.matmul(
                            po[:, :nsz],
                            tC[:, tsl],
                            wo01_sb[:, noff : noff + nsz],
                            start=True,
                            stop=False,
                        )
                        nc.tensor.matmul(
                            po[:, :nsz],
                            tD[:, tsl],
                            wo2_sb[:, noff : noff + nsz],
                            start=False,
                            stop=True,
                        )
                        nc.vector.tensor_copy(
                            out=so[:, noff : noff + nsz], in_=po[:, :nsz]
                        )
                    nc.sync.dma_start(out=out[tsl, :], in_=so[:])

            for qb in range(NQB):
                qsl = slice(qb * QB, (qb + 1) * QB)

                # -- projection chunk qb: columns [qb*512, qb*512+512) --
                xt = xpool.tile([P, 6, QB], F32R, tag="xt", name=f"xt{qb}")
                nc.sync.dma_start(
                    out=xt[:], in_=xT[:, qsl].rearrange("(co p) t -> p co t", p=P)
                )
                for ci in range(3):
                    ps = p1psum.tile([P, QB], F32, tag="p1", name=f"p1_{qb}_{ci}")
                    for c6 in range(6):
                        nc.tensor.matmul(
                            ps[:],
                            wqk_sb[:, c6, ci * P : (ci + 1) * P],
                            xt[:, c6, :],
                            start=(c6 == 0),
                            stop=(c6 == 5),
                        )
                    if ci == 0:
                        # [q_h0 | q_h1]
                        nc.vector.tensor_copy(out=tA[:, qsl], in_=ps[:])
                    elif ci == 1:
                        # [k_h0 | k_h1] -> split into padded kB tiles
                        # (aligned partition ranges; no base crossing)
                        nc.vector.tensor_copy(out=kB[0][0:D, qsl], in_=ps[0:D, :])
                        nc.vector.tensor_copy(out=kB[1][D:P, qsl], in_=ps[D:P, :])
                    else:
                        # [q_h2 | k_h2]: k_h2 lands aligned; q_h2 must
                        # cross to partition base 64 of tD, which only a
                        # DMA can do (engines cannot cross partitions)
                        nc.vector.tensor_copy(out=kB[2][D:P, qsl], in_=ps[D:P, :])
                        stg = xpool.tile([D, QB], F32R, tag="stg")
                        nc.vector.tensor_copy(out=stg[:], in_=ps[0:D, :])
                        nc.sync.dma_start(out=tD[D:P, qsl], in_=stg[:])
                for half in range(QB // P):
                    ktv = qb * (QB // P) + half
                    ps2 = p1psum.tile([P, QB], F32, tag="p1", name=f"p1v_{qb}_{half}")
                    for c6 in range(6):
                        nc.tensor.matmul(
                            ps2[:, 0:256],
                            xt[:, c6, half * P : (half + 1) * P],
                            wv_sb[:, c6, :],
                            start=(c6 == 0),
                            stop=(c6 == 5),
                        )
                    nc.vector.tensor_copy(out=vaug[:, ktv, :], in_=ps2[:, 0:256])
                # restore the ones columns the v copies just overwrote
                for h in range(HPC):
                    nc.vector.tensor_copy(
                        out=vaug[:, qb * (QB // P) : (qb + 1) * (QB // P),
                                 h * (D + 1) + D],
                        in_=ones_f32[:, 0 : QB // P],
                    )

                # -- attention for q block qb --
                # rows 0:64 of tD (outT_h2) are read by h2's padded-K
                # scores before they are first written; zero them so
                # stray NaN/Inf bit patterns cannot poison 0*x products
                nc.vector.tensor_copy(out=tD[0:D, qsl], in_=zeros_f32[0:D, :])
                for h in range(HPC):
                    nkt = 4 * qb + 4
                    outp = opool.tile([D + 1, QB], F32, tag="outT")
                    for kt in range(nkt):
                        ktr = kt - 4 * qb
                        # clamp co to 256: f32r matmuls with moving dim
                        # <256 drop to 1/4 rate, so N=128 costs more
                        # than N=256 (extra columns are masked anyway)
                        co = min(max(0, P * ktr), 256)
                        sp = spool.tile([P, QB], F32, tag="scores")
                        nc.tensor.matmul(
                            sp[:, co:],
                            kB[h][:, kt * P : (kt + 1) * P],
                            qmv(h)[:, qb * QB + co : (qb + 1) * QB],
                            start=True,
                            stop=True,
                        )
                        ex = epool.tile([P, QB], F32R, tag="ex")
                        nc.scalar.activation(
                            out=ex[:, co:],
                            in_=sp[:, co:],
                            func=mybir.ActivationFunctionType.Exp,
                            scale=float(D) ** -0.5,
                        )
                        if ktr >= 0:  # diagonal band: causal mask
                            nc.gpsimd.affine_select(
                                out=ex[:, co:],
                                in_=ex[:, co:],
                                compare_op=mybir.AluOpType.is_ge,
                                fill=0.0,
                                base=co - ktr * P,
                                pattern=[[1, QB - co]],
                                channel_multiplier=-1,
                            )
                        nc.tensor.matmul(
                            outp[:, co:],
                            vaug[:, kt, h * (D + 1) : (h + 1) * (D + 1)],
                            ex[:, co:],
                            start=(kt == 0),
                            stop=(kt == nkt - 1),
                        )
                    # softmax denominators: reciprocal of outp row 64 stays at
                    # partition base 64 (engines cannot cross partitions); a
                    # partition-broadcast DMA then fans it out across 0:64
                    recip = rpool.tile([D + 1, QB], F32, tag="recip", bufs=3)
                    nc.vector.reciprocal(
                        out=recip[D : D + 1, :], in_=outp[D : D + 1, :]
                    )
                    # partition-broadcast via DRAM bounce (SBUF sources must
                    # have nonzero partition step; DRAM reads may broadcast)
                    dsc = dpool.tile([1, QB], F32, tag="dsc")
                    nc.sync.dma_start(out=dsc[:], in_=recip[D : D + 1, :])
                    bcs = rpool.tile([D, QB], F32, tag="bcs", bufs=3)
                    nc.gpsimd.dma_start(
                        out=bcs[:],
                        in_=bass.AP(
                            tensor=dsc.tensor,
                            offset=dsc.offset,
                            ap=[[0, D]] + list(dsc.ap[-1:]),
                        ),
                    )
                    if h == 0:
                        nc.vector.tensor_mul(
                            out=tC[0:D, qsl], in0=outp[0:D, :], in1=bcs[:]
                        )
                    elif h == 2:
                        nc.vector.tensor_mul(
                            out=tD[0:D, qsl], in0=outp[0:D, :], in1=bcs[:]
                        )
                    else:
                        # h1 lives at partition base 64 of tC; engines cannot
                        # cross partitions, so normalize into a staging tile
                        # and DMA-bounce it up
                        ot = rpool.tile([D, QB], F32R, tag="otmp", bufs=2)
                        nc.vector.tensor_mul(
                            out=ot[:], in0=outp[0:D, :], in1=bcs[:]
                        )
                        nc.sync.dma_start(out=tC[D:P, qsl], in_=ot[:])

                # deferred output projection: emit block qb-1 now so its
                # normalize chain resolved during this block's attention
                if qb > 0:
                    emit_outproj(qb - 1)
            emit_outproj(NQB - 1)


def _get_nc():
    if "nc" not in _CACHE:
        _CACHE["nc"] = _build_nc()
    return _CACHE["nc"]


def _shard_inputs(x, w_qkv, w_out):
    """Build per-core input maps."""
    x = np.asarray(x, dtype=np.float32)
    w_qkv = np.asarray(w_qkv, dtype=np.float32)
    w_out = np.asarray(w_out, dtype=np.float32)
    xTs = [np.ascontiguousarray(x[b].T) for b in range(B)]
    in_maps = []
    for c in range(NCORES):
        b = c // 4
        heads = [HPC * (c % 4) + i for i in range(HPC)]
        q = [w_qkv[:, h * D : (h + 1) * D] for h in heads]
        k = [w_qkv[:, C + h * D : C + (h + 1) * D] for h in heads]
        wqk = np.concatenate([q[0], q[1], k[0], k[1], q[2], k[2]], axis=1)
        wv = np.zeros((C, 256), dtype=np.float32)
        for i, h in enumerate(heads):
            wv[:, i * (D + 1) : i * (D + 1) + D] = w_qkv[
                :, 2 * C + h * D : 2 * C + (h + 1) * D
            ]
        # [h0 rows | h1 rows | h2 rows | 64 zero rows]: rows 192:256
        # annihilate the qT_h2 junk in rows 64:128 of tD during the
        # output projection's second (tD) contraction
        wo = np.concatenate(
            [w_out[h * D : (h + 1) * D, :] for h in heads]
            + [np.zeros((D, C), dtype=np.float32)],
            axis=0,
        )
        in_maps.append(
            {
                "xT": xTs[b],
                "wqk": np.ascontiguousarray(wqk),
                "wv": wv,
                "wo": np.ascontiguousarray(wo),
            }
        )
    return in_maps


def kernel(x, w_qkv, w_out, b_out):
    nc = _get_nc()
    in_maps = _shard_inputs(x, w_qkv, w_out)
    res = run_bass_kernel_spmd(nc, in_maps, core_ids=list(range(NCORES)))
    b_out = np.asarray(b_out, dtype=np.float32)
    outs = []
    for b in range(B):
        acc = res.results[4 * b]["out"].astype(np.float32).copy()
        for c in range(4 * b + 1, 4 * b + 4):
            acc += res.results[c]["out"]
        outs.append(acc + b_out[None, :])
    return np.stack(outs, axis=0)
